# revision 10
# baseline (speedup 1.0000x reference)
"""Trainium2 Bass kernel for batched 16-head attention (B=8, N=1024, D=1024).

Sharding: data-parallel over batch — one batch element per NeuronCore (8 cores).

Per-core pipeline (all matmuls in float32r, fp32 storage):
  1. q,k projected feature-major (qT/kT = W.T @ x.T), v projected seq-major,
     with a ones-column appended per head so the attention-weight row-sums
     (softmax denominators) fall out of the same matmul as out = e.T @ v_aug.
  2. scores computed transposed (scores_T[j, i] = k . q) so the mask penalty
     is a per-partition bias fused into the ScalarE exp together with the
     1/sqrt(d) scale: e = exp(0.125 * scores_T + pen[j]).
  3. masked key rows are dropped entirely (host gathers kept rows; a masked
     row's exp(-10000 + s) is exactly 0.0 in f32, so dropping it is exact).
  4. out_raw_T accumulated per head, PE-transposed back to natural layout,
     normalized by the row-sums, DMA'd out.
"""

import sys

sys.path.insert(0, "/opt/trn_rl_repo")

import numpy as np

import concourse.bass as bass
import concourse.bacc as bacc
import concourse.mybir as mybir
from concourse.tile import TileContext
from concourse.bass_utils import run_bass_kernel_spmd

B = 8
N = 1024          # sequence length (queries)
D = 1024          # model dim
H = 16            # heads
DH = 64           # head dim
NPAIR = H // 2    # head pairs (2 heads share one 128-row feature tile)
P = 128
F32 = mybir.dt.float32
F32R = mybir.dt.float32r
EXP = mybir.ActivationFunctionType.Exp

_CACHE = {}


def build_nc(n_j, repeat=0):
    """Build the per-core Bass graph.

    n_j: padded count of kept key rows (multiple of 128). If n_j == N the
         k/v projections read the full xT input (no separate gathered input).
    repeat: if > 0, wrap the whole compute in a For_i timing loop.

    Structure: projection matmul chains are interleaved into the attention
    jc-loops via a work feeder, so PE fills its exp-latency gaps with proj
    work instead of stalling (PE executes strictly in emission order).
    """
    n_jc = n_j // 128
    share_xt = n_j == N

    nc = bacc.Bacc(None, target_bir_lowering=False)
    xt_ext = nc.declare_dram_parameter("xt", [D, N], F32, isOutput=False)
    if not share_xt:
        xtkv_ext = nc.declare_dram_parameter("xtkv", [D, n_j], F32, isOutput=False)
    w_ext = nc.declare_dram_parameter("w", [D, 3 * D], F32, isOutput=False)
    pen_ext = nc.declare_dram_parameter("pen", [P, n_jc], F32, isOutput=False)
    id_ext = nc.declare_dram_parameter("ident", [P, P], F32, isOutput=False)
    out_ext = nc.declare_dram_parameter("out", [N, D], F32, isOutput=True)

    with TileContext(nc) as tc:
        with (
            tc.tile_pool(name="const", bufs=1) as const_pool,
            tc.tile_pool(name="xt", bufs=1) as xt_pool,
            tc.tile_pool(name="qk", bufs=1) as qk_pool,
            tc.tile_pool(name="vnat", bufs=1) as v_pool,
            tc.tile_pool(name="wq", bufs=3) as w_pool,
            tc.tile_pool(name="wv", bufs=1) as wv_pool,
            tc.tile_pool(name="e", bufs=2) as e_pool,
            tc.tile_pool(name="ot", bufs=2) as ot_pool,
            tc.tile_pool(name="ob", bufs=3) as ob_pool,
            tc.tile_pool(name="pss", bufs=1, space="PSUM") as pss_pool,
            tc.tile_pool(name="pso", bufs=1, space="PSUM") as pso_pool,
            tc.tile_pool(name="psj", bufs=2, space="PSUM") as psj_pool,
        ):
            pen_sb = const_pool.tile([P, n_jc], F32, tag="pen")
            nc.sync.dma_start(out=pen_sb[:], in_=pen_ext[:])
            id_sb = const_pool.tile([P, P], F32, tag="ident")
            nc.sync.dma_start(out=id_sb[:], in_=id_ext[:])

            xt_sb = [xt_pool.tile([P, N], F32R, tag=f"xt{dc}", name=f"xt{dc}")
                     for dc in range(8)]
            for h0 in range(0, N, 512):
                for dc in range(8):
                    nc.sync.dma_start(
                        out=xt_sb[dc][:, h0:h0 + 512],
                        in_=xt_ext[dc * P:(dc + 1) * P, h0:h0 + 512].bitcast(F32R),
                    )
                if h0 == 0:
                    # first q/k chains' weights right after the first xt halves
                    w_pre = {}
                    for fc_ in (0, 8):
                        wt_ = w_pool.tile([P, D], F32R, tag="w", name=f"wpre{fc_}")
                        for dc_ in range(8):
                            nc.sync.dma_start(
                                out=wt_[:, dc_ * P:(dc_ + 1) * P],
                                in_=w_ext[dc_ * P:(dc_ + 1) * P,
                                          fc_ * P:(fc_ + 1) * P].bitcast(F32R),
                            )
                        w_pre[fc_] = wt_
            if share_xt:
                xtkv_sb = xt_sb
            else:
                xtkv_sb = []
                for dc in range(8):
                    t = xt_pool.tile([P, n_j], F32R, tag=f"xtkv{dc}")
                    nc.sync.dma_start(
                        out=t[:, :], in_=xtkv_ext[dc * P:(dc + 1) * P, :].bitcast(F32R)
                    )
                    xtkv_sb.append(t)

            def body():
                qk_sb = [None] * 16
                v_nat = []
                for jc in range(n_jc):
                    t = v_pool.tile([P, H * 65], F32R, tag=f"v{jc}", name=f"v{jc}")
                    nc.vector.memset(
                        t.rearrange("p (h c) -> p h c", c=65)[:, :, 64:65].bitcast(F32),
                        1.0,
                    )
                    v_nat.append(t)
                wv_sb = {}

                # ---------- projection work units ----------
                def qk_chain(fc):
                    """Yield one closure per PE matmul for projection chain fc."""
                    n_cols = N if fc < 8 else n_j
                    src_ = xt_sb if fc < 8 else xtkv_sb
                    state = {}

                    def first():
                        if fc in w_pre:
                            w_sb = w_pre[fc]
                        else:
                            w_sb = w_pool.tile([P, D], F32R, tag="w", name=f"w{fc}")
                            for dc_ in range(8):
                                nc.sync.dma_start(
                                    out=w_sb[:, dc_ * P:(dc_ + 1) * P],
                                    in_=w_ext[dc_ * P:(dc_ + 1) * P,
                                              fc * P:(fc + 1) * P].bitcast(F32R),
                                )
                        ps = psj_pool.tile([P, N], F32, tag="proj", name=f"pj{fc}")
                        state["w"] = w_sb
                        state["ps"] = ps

                    halves = [(c0, min(c0 + 512, n_cols)) for c0 in range(0, n_cols, 512)]
                    units = [(hi, dc) for hi in range(len(halves)) for dc in range(8)]

                    def make(i, hi, dc):
                        def emit():
                            if i == 0:
                                first()
                            c0, c1 = halves[hi]
                            nc.tensor.matmul(
                                state["ps"][:, c0:c1],
                                state["w"][:, dc * P:(dc + 1) * P],
                                src_[dc][:, c0:c1],
                                start=(dc == 0), stop=(dc == 7),
                            )
                            if i == len(units) - 1:
                                dst = qk_pool.tile([P, n_cols], F32R,
                                                   tag=f"qk{fc}", name=f"qk{fc}")
                                nc.vector.tensor_copy(dst[:], state["ps"][:, :n_cols])
                                qk_sb[fc] = dst
                        return emit
                    return [make(i, hi, dc) for i, (hi, dc) in enumerate(units)]

                def wv_dma(hv):
                    def emit():
                        tiles = []
                        for dc_ in range(8):
                            t = wv_pool.tile([P, 512], F32R, tag=f"wv{hv}_{dc_}",
                                             name=f"wv{hv}_{dc_}")
                            nc.sync.dma_start(
                                out=t[:],
                                in_=w_ext[dc_ * P:(dc_ + 1) * P,
                                          2048 + hv * 512:2048 + (hv + 1) * 512
                                          ].bitcast(F32R),
                            )
                            tiles.append(t)
                        wv_sb[hv] = tiles
                    return emit

                def v_chain(hv, jc):
                    def make(dc):
                        def emit():
                            if dc == 0:
                                ps = psj_pool.tile([P, 512], F32, tag="proj",
                                                   name=f"pv{hv}_{jc}")
                                v_chain.ps = ps
                            nc.tensor.matmul(
                                v_chain.ps[:],
                                xtkv_sb[dc][:, jc * P:(jc + 1) * P],
                                wv_sb[hv][dc][:],
                                start=(dc == 0), stop=(dc == 7),
                            )
                            if dc == 7:
                                dstv = v_nat[jc].rearrange("p (h c) -> p h c", c=65)
                                nc.vector.tensor_copy(
                                    dstv[:, hv * 8:(hv + 1) * 8, 0:64],
                                    v_chain.ps[:].rearrange("p (h c) -> p h c", c=64),
                                )
                        return emit
                    return [make(dc) for dc in range(8)]

                # ---------- upfront: q0, k0, v(hv=0) ----------
                wv_dma(0)()
                for u in qk_chain(0):
                    u()
                for u in qk_chain(8):
                    u()
                for jc in range(n_jc):
                    for u in v_chain(0, jc):
                        u()

                # ---------- stream: remaining proj work, fed into attention ----
                stream = []
                markers = {}          # pair -> index into stream that must be done
                stream.append(wv_dma(1))
                # v(hv=1) chains must all be emitted before pair NPAIR//2
                # (the first consumer of head slots 8-15): spread over p=1..3.
                vq = list(range(n_jc))
                per_p = -(-len(vq) // 3)
                for p in range(1, NPAIR):
                    stream.extend(qk_chain(p))
                    stream.extend(qk_chain(8 + p))
                    if p <= 3:
                        for jc in vq[(p - 1) * per_p:p * per_p]:
                            stream.extend(v_chain(1, jc))
                    markers[p] = len(stream)
                pos = [0]
                extra = []            # epilogue PE units (fill work, no deadline)

                def feed(k):
                    done = 0
                    while done < k:
                        if pos[0] < len(stream):
                            stream[pos[0]]()
                            pos[0] += 1
                        elif extra:
                            extra.pop(0)()
                        else:
                            return
                        done += 1

                def feed_until(idx):
                    while pos[0] < idx:
                        stream[pos[0]]()
                        pos[0] += 1

                # ---------- attention ----------
                def epilogue_units(p, ih, ot):
                    i0 = ih * 512
                    state = {}

                    def tr(ic, half):
                        def emit():
                            if "ps" not in state:
                                state["ps"] = psj_pool.tile(
                                    [P, 1024], F32, tag="proj", name=f"pt{p}_{ih}")
                            ps_t = state["ps"]
                            c = ic * P
                            t0 = ic * 256
                            if half == 0:
                                nc.tensor.transpose(
                                    ps_t[:, t0:t0 + 65],
                                    ot[:, c:c + P], id_sb[0:65, 0:65])
                            else:
                                nc.tensor.transpose(
                                    ps_t[:, t0 + 128:t0 + 193],
                                    ot[:, 512 + c:512 + c + P], id_sb[0:65, 0:65])
                            if ic == 3 and half == 1:
                                tail(ps_t)
                        return emit

                    def tail(ps_t):
                        for ic in range(4):
                            t0 = ic * 256
                            ob = ob_pool.tile([P, P], F32, tag="ob",
                                              name=f"ob{p}_{ih}_{ic}")
                            rc = ob_pool.tile([P, 2], F32, tag="rc",
                                              name=f"rc{p}_{ih}_{ic}")
                            nc.vector.reciprocal(rc[:, 0:1], ps_t[:, t0 + 64:t0 + 65])
                            nc.vector.reciprocal(rc[:, 1:2], ps_t[:, t0 + 192:t0 + 193])
                            nc.vector.tensor_scalar_mul(
                                ob[:, 0:64], ps_t[:, t0:t0 + 64], rc[:, 0:1]
                            )
                            nc.vector.tensor_scalar_mul(
                                ob[:, 64:128], ps_t[:, t0 + 128:t0 + 192], rc[:, 1:2]
                            )
                            nc.sync.dma_start(
                                out=out_ext[i0 + ic * P:i0 + (ic + 1) * P,
                                            p * P:(p + 1) * P],
                                in_=ob[:],
                            )

                    return [tr(ic, h) for ic in range(4) for h in range(2)]

                for p in range(NPAIR):
                    if p in markers:
                        feed_until(markers[p])
                    qT = qk_sb[p]
                    kT = qk_sb[8 + p]
                    ha, hb = 2 * p, 2 * p + 1
                    for ih in range(2):
                        i0 = ih * 512
                        ps_s = pss_pool.tile([P, 1024], F32, tag="s",
                                             name=f"s{p}_{ih}")
                        ps_o = pso_pool.tile([65, 1024], F32, tag="o",
                                             name=f"o{p}_{ih}")
                        for jc in range(n_jc):
                            nc.tensor.matmul(
                                ps_s[:, 0:512],
                                kT[0:64, jc * P:(jc + 1) * P],
                                qT[0:64, i0:i0 + 512],
                                start=True, stop=True,
                            )
                            nc.tensor.matmul(
                                ps_s[:, 512:1024],
                                kT[64:128, jc * P:(jc + 1) * P],
                                qT[64:128, i0:i0 + 512],
                                start=True, stop=True,
                            )
                            e_sb = e_pool.tile([P, 1024], F32R, tag="e",
                                               name=f"e{p}_{ih}_{jc}")
                            nc.scalar.activation(
                                e_sb[:], ps_s[:], EXP,
                                bias=pen_sb[:, jc:jc + 1], scale=0.125,
                            )
                            nc.tensor.matmul(
                                ps_o[:, 0:512],
                                v_nat[jc][:, ha * 65:(ha + 1) * 65],
                                e_sb[:, 0:512],
                                start=(jc == 0), stop=(jc == n_jc - 1),
                            )
                            nc.tensor.matmul(
                                ps_o[:, 512:1024],
                                v_nat[jc][:, hb * 65:(hb + 1) * 65],
                                e_sb[:, 512:1024],
                                start=(jc == 0), stop=(jc == n_jc - 1),
                            )
                            feed(3)
                        ot = ot_pool.tile([65, 1024], F32, tag="ot",
                                          name=f"ot{p}_{ih}")
                        nc.vector.tensor_copy(ot[:], ps_o[:])
                        extra.extend(epilogue_units(p, ih, ot))
                feed(10 ** 9)
                while extra:
                    extra.pop(0)()

            if repeat > 0:
                with tc.For_i(0, repeat, 1):
                    body()
            else:
                body()

    nc.compile()
    return nc


def _host_prep(x, mask, w_qkv):
    """Shard + lay out inputs per core. Returns (in_maps, n_j)."""
    x = np.ascontiguousarray(x, dtype=np.float32)
    mask = np.asarray(mask)
    w_qkv = np.ascontiguousarray(w_qkv, dtype=np.float32)

    # kept key rows per batch: j=0 always kept, then mask over rows 1..N-1
    keep = np.concatenate([np.ones((B, 1), dtype=bool), mask.astype(bool)], axis=1)
    counts = keep.sum(axis=1)
    n_j = int(np.ceil(counts.max() / 128.0) * 128)
    n_j = min(n_j, N)

    ident = np.eye(P, dtype=np.float32)
    in_maps = []
    for b in range(B):
        xt = np.ascontiguousarray(x[b].T)               # [D, N]
        idx = np.nonzero(keep[b])[0]
        m = {"xt": xt, "w": w_qkv, "ident": ident}
        pen = np.full(n_j, -10000.0, dtype=np.float32)  # padding rows masked out
        pen[: len(idx)] = 0.0
        m["pen"] = np.ascontiguousarray(pen.reshape(n_j // 128, 128).T)  # [128, n_jc]
        if n_j == N:
            # no gather: full rows, penalty by original position
            penf = np.full(N, -10000.0, dtype=np.float32)
            penf[keep[b]] = 0.0
            m["pen"] = np.ascontiguousarray(penf.reshape(N // 128, 128).T)
        else:
            xkv = np.zeros((D, n_j), dtype=np.float32)
            xkv[:, : len(idx)] = xt[:, idx]
            m["xtkv"] = xkv
        in_maps.append(m)
    return in_maps, n_j


def kernel(x, mask, w_qkv):
    in_maps, n_j = _host_prep(x, mask, w_qkv)
    if n_j not in _CACHE:
        _CACHE[n_j] = build_nc(n_j)
    nc = _CACHE[n_j]
    res = run_bass_kernel_spmd(nc, in_maps, core_ids=list(range(B)))
    out = np.stack([np.asarray(res.results[i]["out"]) for i in range(B)], axis=0)
    return out.astype(np.float32)


if __name__ == "__main__":
    rng = np.random.default_rng(0)
    x = rng.standard_normal((B, N, D), dtype=np.float32)
    mask = rng.integers(0, 2, size=(B, N - 1)).astype(np.int32)
    w = (rng.standard_normal((D, 3 * D), dtype=np.float32) * D ** -0.5).astype(np.float32)
    out = kernel(x=x, mask=mask, w_qkv=w)
    print("out", out.shape, out.dtype, float(np.abs(out).mean()))


# revision 11
# speedup vs baseline: 1.4575x; 1.4575x over previous
"""Trainium2 Bass kernel for batched 16-head attention (B=8, N=1024, D=1024).

Sharding: data-parallel over batch — one batch element per NeuronCore (8 cores).

Per-core pipeline (all matmuls in float32r, fp32 storage):
  1. q,k projected feature-major (qT/kT = W.T @ x.T), v projected seq-major,
     with a ones-column appended per head so the attention-weight row-sums
     (softmax denominators) fall out of the same matmul as out = e.T @ v_aug.
  2. scores computed transposed (scores_T[j, i] = k . q) so the mask penalty
     is a per-partition bias fused into the ScalarE exp together with the
     1/sqrt(d) scale: e = exp(0.125 * scores_T + pen[j]).
  3. masked key rows are dropped entirely (host gathers kept rows; a masked
     row's exp(-10000 + s) is exactly 0.0 in f32, so dropping it is exact).
  4. out_raw_T accumulated per head, PE-transposed back to natural layout,
     normalized by the row-sums, DMA'd out.
"""

import sys

sys.path.insert(0, "/opt/trn_rl_repo")

import numpy as np

import concourse.bass as bass
import concourse.bacc as bacc
import concourse.mybir as mybir
from concourse.tile import TileContext
from concourse.bass_utils import run_bass_kernel_spmd

B = 8
N = 1024          # sequence length (queries)
D = 1024          # model dim
H = 16            # heads
DH = 64           # head dim
NPAIR = H // 2    # head pairs (2 heads share one 128-row feature tile)
P = 128
F32 = mybir.dt.float32
F32R = mybir.dt.float32r
EXP = mybir.ActivationFunctionType.Exp

_CACHE = {}


def build_nc(n_j, repeat=0):
    """Build the per-core Bass graph.

    n_j: padded count of kept key rows (multiple of 128). If n_j == N the
         k/v projections read the full xT input (no separate gathered input).
    repeat: if > 0, wrap the whole compute in a For_i timing loop.

    Structure: projection matmul chains are interleaved into the attention
    jc-loops via a work feeder, so PE fills its exp-latency gaps with proj
    work instead of stalling (PE executes strictly in emission order).
    """
    n_jc = n_j // 128
    share_xt = n_j == N

    nc = bacc.Bacc(None, target_bir_lowering=False)
    xt_ext = nc.declare_dram_parameter("xt", [D, N], F32, isOutput=False)
    if not share_xt:
        xtkv_ext = nc.declare_dram_parameter("xtkv", [D, n_j], F32, isOutput=False)
    w_ext = nc.declare_dram_parameter("w", [D, 3 * D], F32, isOutput=False)
    pen_ext = nc.declare_dram_parameter("pen", [P, n_jc], F32, isOutput=False)
    id_ext = nc.declare_dram_parameter("ident", [P, P], F32, isOutput=False)
    out_ext = nc.declare_dram_parameter("out", [N, D], F32, isOutput=True)

    with TileContext(nc) as tc:
        with (
            tc.tile_pool(name="const", bufs=1) as const_pool,
            tc.tile_pool(name="xt", bufs=1) as xt_pool,
            tc.tile_pool(name="qk", bufs=1) as qk_pool,
            tc.tile_pool(name="vnat", bufs=1) as v_pool,
            tc.tile_pool(name="wq", bufs=3) as w_pool,
            tc.tile_pool(name="wv", bufs=1) as wv_pool,
            tc.tile_pool(name="e", bufs=2) as e_pool,
            tc.tile_pool(name="ot", bufs=4) as ot_pool,
            tc.tile_pool(name="ob", bufs=3) as ob_pool,
            tc.tile_pool(name="pss", bufs=1, space="PSUM") as pss_pool,
            tc.tile_pool(name="pso", bufs=1, space="PSUM") as pso_pool,
            tc.tile_pool(name="psj", bufs=2, space="PSUM") as psj_pool,
        ):
            pen_sb = const_pool.tile([P, n_jc], F32, tag="pen")
            nc.sync.dma_start(out=pen_sb[:], in_=pen_ext[:])
            id_sb = const_pool.tile([P, P], F32, tag="ident")
            nc.sync.dma_start(out=id_sb[:], in_=id_ext[:])

            xt_sb = [xt_pool.tile([P, N], F32R, tag=f"xt{dc}", name=f"xt{dc}")
                     for dc in range(8)]
            for h0 in range(0, N, 512):
                for dc in range(8):
                    nc.sync.dma_start(
                        out=xt_sb[dc][:, h0:h0 + 512],
                        in_=xt_ext[dc * P:(dc + 1) * P, h0:h0 + 512].bitcast(F32R),
                    )
                if h0 == 0:
                    # first q/k chains' weights right after the first xt halves
                    w_pre = {}
                    for fc_ in (0, 8):
                        wt_ = w_pool.tile([P, D], F32R, tag="w", name=f"wpre{fc_}")
                        for dc_ in range(8):
                            nc.sync.dma_start(
                                out=wt_[:, dc_ * P:(dc_ + 1) * P],
                                in_=w_ext[dc_ * P:(dc_ + 1) * P,
                                          fc_ * P:(fc_ + 1) * P].bitcast(F32R),
                            )
                        w_pre[fc_] = wt_
            if share_xt:
                xtkv_sb = xt_sb
            else:
                xtkv_sb = []
                for dc in range(8):
                    t = xt_pool.tile([P, n_j], F32R, tag=f"xtkv{dc}")
                    nc.sync.dma_start(
                        out=t[:, :], in_=xtkv_ext[dc * P:(dc + 1) * P, :].bitcast(F32R)
                    )
                    xtkv_sb.append(t)

            def body():
                qk_sb = [None] * 16
                v_nat = []
                for jc in range(n_jc):
                    t = v_pool.tile([P, H * 65], F32R, tag=f"v{jc}", name=f"v{jc}")
                    nc.vector.memset(
                        t.rearrange("p (h c) -> p h c", c=65)[:, :, 64:65].bitcast(F32),
                        1.0,
                    )
                    v_nat.append(t)
                wv_sb = {}

                # ---------- projection work units ----------
                def qk_chain(fc):
                    """Yield one closure per PE matmul for projection chain fc."""
                    n_cols = N if fc < 8 else n_j
                    src_ = xt_sb if fc < 8 else xtkv_sb
                    state = {}

                    def first():
                        if fc in w_pre:
                            w_sb = w_pre[fc]
                        else:
                            w_sb = w_pool.tile([P, D], F32R, tag="w", name=f"w{fc}")
                            for dc_ in range(8):
                                nc.sync.dma_start(
                                    out=w_sb[:, dc_ * P:(dc_ + 1) * P],
                                    in_=w_ext[dc_ * P:(dc_ + 1) * P,
                                              fc * P:(fc + 1) * P].bitcast(F32R),
                                )
                        ps = psj_pool.tile([P, N], F32, tag="proj", name=f"pj{fc}")
                        state["w"] = w_sb
                        state["ps"] = ps

                    halves = [(c0, min(c0 + 512, n_cols)) for c0 in range(0, n_cols, 512)]
                    units = [(hi, dc) for hi in range(len(halves)) for dc in range(8)]

                    def make(i, hi, dc):
                        def emit():
                            if i == 0:
                                first()
                            c0, c1 = halves[hi]
                            nc.tensor.matmul(
                                state["ps"][:, c0:c1],
                                state["w"][:, dc * P:(dc + 1) * P],
                                src_[dc][:, c0:c1],
                                start=(dc == 0), stop=(dc == 7),
                            )
                            if i == len(units) - 1:
                                dst = qk_pool.tile([P, n_cols], F32R,
                                                   tag=f"qk{fc}", name=f"qk{fc}")
                                nc.vector.tensor_copy(dst[:], state["ps"][:, :n_cols])
                                qk_sb[fc] = dst
                        return emit
                    return [make(i, hi, dc) for i, (hi, dc) in enumerate(units)]

                def wv_dma(hv):
                    def emit():
                        tiles = []
                        for dc_ in range(8):
                            t = wv_pool.tile([P, 512], F32R, tag=f"wv{hv}_{dc_}",
                                             name=f"wv{hv}_{dc_}")
                            nc.sync.dma_start(
                                out=t[:],
                                in_=w_ext[dc_ * P:(dc_ + 1) * P,
                                          2048 + hv * 512:2048 + (hv + 1) * 512
                                          ].bitcast(F32R),
                            )
                            tiles.append(t)
                        wv_sb[hv] = tiles
                    return emit

                def v_chain(hv, jc):
                    def make(dc):
                        def emit():
                            if dc == 0:
                                ps = psj_pool.tile([P, 512], F32, tag="proj",
                                                   name=f"pv{hv}_{jc}")
                                v_chain.ps = ps
                            nc.tensor.matmul(
                                v_chain.ps[:],
                                xtkv_sb[dc][:, jc * P:(jc + 1) * P],
                                wv_sb[hv][dc][:],
                                start=(dc == 0), stop=(dc == 7),
                            )
                            if dc == 7:
                                dstv = v_nat[jc].rearrange("p (h c) -> p h c", c=65)
                                nc.vector.tensor_copy(
                                    dstv[:, hv * 8:(hv + 1) * 8, 0:64],
                                    v_chain.ps[:].rearrange("p (h c) -> p h c", c=64),
                                )
                        return emit
                    return [make(dc) for dc in range(8)]

                # ---------- upfront: q0, k0, v(hv=0) ----------
                wv_dma(0)()
                for u in qk_chain(0):
                    u()
                for u in qk_chain(8):
                    u()
                for jc in range(n_jc):
                    for u in v_chain(0, jc):
                        u()

                # ---------- stream: remaining proj work, fed into attention ----
                stream = []
                markers = {}          # pair -> index into stream that must be done
                stream.append(wv_dma(1))
                # v(hv=1) chains must all be emitted before pair NPAIR//2
                # (the first consumer of head slots 8-15): spread over p=1..3.
                vq = list(range(n_jc))
                per_p = -(-len(vq) // 3)
                for p in range(1, NPAIR):
                    stream.extend(qk_chain(p))
                    stream.extend(qk_chain(8 + p))
                    if p <= 3:
                        for jc in vq[(p - 1) * per_p:p * per_p]:
                            stream.extend(v_chain(1, jc))
                    markers[p] = len(stream)
                pos = [0]
                extra = []            # epilogue PE units (fill work, no deadline)

                def feed(k, ne=0):
                    done = 0
                    while done < ne and extra:
                        extra.pop(0)()
                        done += 1
                    while done < k + ne:
                        if pos[0] < len(stream):
                            stream[pos[0]]()
                            pos[0] += 1
                        elif extra:
                            extra.pop(0)()
                        else:
                            return
                        done += 1

                def feed_until(idx):
                    while pos[0] < idx:
                        stream[pos[0]]()
                        pos[0] += 1

                # ---------- attention ----------
                def epilogue_units(p, ih, ot):
                    i0 = ih * 512
                    state = {}

                    def tr(ic, half):
                        def emit():
                            if "ps" not in state:
                                state["ps"] = psj_pool.tile(
                                    [P, 1024], F32, tag="proj", name=f"pt{p}_{ih}")
                            ps_t = state["ps"]
                            c = ic * P
                            t0 = ic * 256
                            if half == 0:
                                nc.tensor.transpose(
                                    ps_t[:, t0:t0 + 65],
                                    ot[:, c:c + P], id_sb[0:65, 0:65])
                            else:
                                nc.tensor.transpose(
                                    ps_t[:, t0 + 128:t0 + 193],
                                    ot[:, 512 + c:512 + c + P], id_sb[0:65, 0:65])
                            if ic == 3 and half == 1:
                                tail(ps_t)
                        return emit

                    def tail(ps_t):
                        for ic in range(4):
                            t0 = ic * 256
                            ob = ob_pool.tile([P, P], F32, tag="ob",
                                              name=f"ob{p}_{ih}_{ic}")
                            rc = ob_pool.tile([P, 2], F32, tag="rc",
                                              name=f"rc{p}_{ih}_{ic}")
                            nc.vector.reciprocal(rc[:, 0:1], ps_t[:, t0 + 64:t0 + 65])
                            nc.vector.reciprocal(rc[:, 1:2], ps_t[:, t0 + 192:t0 + 193])
                            nc.vector.tensor_scalar_mul(
                                ob[:, 0:64], ps_t[:, t0:t0 + 64], rc[:, 0:1]
                            )
                            nc.vector.tensor_scalar_mul(
                                ob[:, 64:128], ps_t[:, t0 + 128:t0 + 192], rc[:, 1:2]
                            )
                            nc.sync.dma_start(
                                out=out_ext[i0 + ic * P:i0 + (ic + 1) * P,
                                            p * P:(p + 1) * P],
                                in_=ob[:],
                            )

                    return [tr(ic, h) for ic in range(4) for h in range(2)]

                for p in range(NPAIR):
                    if p in markers:
                        feed_until(markers[p])
                    qT = qk_sb[p]
                    kT = qk_sb[8 + p]
                    ha, hb = 2 * p, 2 * p + 1
                    for ih in range(2):
                        i0 = ih * 512
                        ps_s = pss_pool.tile([P, 1024], F32, tag="s",
                                             name=f"s{p}_{ih}")
                        ps_o = pso_pool.tile([65, 1024], F32, tag="o",
                                             name=f"o{p}_{ih}")
                        for jc in range(n_jc):
                            nc.tensor.matmul(
                                ps_s[:, 0:512],
                                kT[0:64, jc * P:(jc + 1) * P],
                                qT[0:64, i0:i0 + 512],
                                start=True, stop=True,
                            )
                            nc.tensor.matmul(
                                ps_s[:, 512:1024],
                                kT[64:128, jc * P:(jc + 1) * P],
                                qT[64:128, i0:i0 + 512],
                                start=True, stop=True,
                            )
                            e_sb = e_pool.tile([P, 1024], F32R, tag="e",
                                               name=f"e{p}_{ih}_{jc}")
                            nc.scalar.activation(
                                e_sb[:], ps_s[:], EXP,
                                bias=pen_sb[:, jc:jc + 1], scale=0.125,
                            )
                            nc.tensor.matmul(
                                ps_o[:, 0:512],
                                v_nat[jc][:, ha * 65:(ha + 1) * 65],
                                e_sb[:, 0:512],
                                start=(jc == 0), stop=(jc == n_jc - 1),
                            )
                            nc.tensor.matmul(
                                ps_o[:, 512:1024],
                                v_nat[jc][:, hb * 65:(hb + 1) * 65],
                                e_sb[:, 512:1024],
                                start=(jc == 0), stop=(jc == n_jc - 1),
                            )
                            feed(3, ne=2)
                        ot = ot_pool.tile([65, 1024], F32, tag="ot",
                                          name=f"ot{p}_{ih}")
                        nc.vector.tensor_copy(ot[:], ps_o[:])
                        extra.extend(epilogue_units(p, ih, ot))
                feed(10 ** 9)
                while extra:
                    extra.pop(0)()

            if repeat > 0:
                with tc.For_i(0, repeat, 1):
                    body()
            else:
                body()

    nc.compile()
    return nc


def _host_prep(x, mask, w_qkv):
    """Shard + lay out inputs per core. Returns (in_maps, n_j)."""
    x = np.ascontiguousarray(x, dtype=np.float32)
    mask = np.asarray(mask)
    w_qkv = np.ascontiguousarray(w_qkv, dtype=np.float32)

    # kept key rows per batch: j=0 always kept, then mask over rows 1..N-1
    keep = np.concatenate([np.ones((B, 1), dtype=bool), mask.astype(bool)], axis=1)
    counts = keep.sum(axis=1)
    n_j = int(np.ceil(counts.max() / 128.0) * 128)
    n_j = min(n_j, N)

    ident = np.eye(P, dtype=np.float32)
    in_maps = []
    for b in range(B):
        xt = np.ascontiguousarray(x[b].T)               # [D, N]
        idx = np.nonzero(keep[b])[0]
        m = {"xt": xt, "w": w_qkv, "ident": ident}
        pen = np.full(n_j, -10000.0, dtype=np.float32)  # padding rows masked out
        pen[: len(idx)] = 0.0
        m["pen"] = np.ascontiguousarray(pen.reshape(n_j // 128, 128).T)  # [128, n_jc]
        if n_j == N:
            # no gather: full rows, penalty by original position
            penf = np.full(N, -10000.0, dtype=np.float32)
            penf[keep[b]] = 0.0
            m["pen"] = np.ascontiguousarray(penf.reshape(N // 128, 128).T)
        else:
            xkv = np.zeros((D, n_j), dtype=np.float32)
            xkv[:, : len(idx)] = xt[:, idx]
            m["xtkv"] = xkv
        in_maps.append(m)
    return in_maps, n_j


def kernel(x, mask, w_qkv):
    in_maps, n_j = _host_prep(x, mask, w_qkv)
    if n_j not in _CACHE:
        _CACHE[n_j] = build_nc(n_j)
    nc = _CACHE[n_j]
    res = run_bass_kernel_spmd(nc, in_maps, core_ids=list(range(B)))
    out = np.stack([np.asarray(res.results[i]["out"]) for i in range(B)], axis=0)
    return out.astype(np.float32)


if __name__ == "__main__":
    rng = np.random.default_rng(0)
    x = rng.standard_normal((B, N, D), dtype=np.float32)
    mask = rng.integers(0, 2, size=(B, N - 1)).astype(np.int32)
    w = (rng.standard_normal((D, 3 * D), dtype=np.float32) * D ** -0.5).astype(np.float32)
    out = kernel(x=x, mask=mask, w_qkv=w)
    print("out", out.shape, out.dtype, float(np.abs(out).mean()))


# revision 12
# speedup vs baseline: 1.7535x; 1.2031x over previous
"""Trainium2 Bass kernel for batched 16-head attention (B=8, N=1024, D=1024).

Sharding: data-parallel over batch — one batch element per NeuronCore (8 cores).

Per-core pipeline (all matmuls in float32r, fp32 storage):
  1. q,k projected feature-major (qT/kT = W.T @ x.T), v projected seq-major,
     with a ones-column appended per head so the attention-weight row-sums
     (softmax denominators) fall out of the same matmul as out = e.T @ v_aug.
  2. scores computed transposed (scores_T[j, i] = k . q) so the mask penalty
     is a per-partition bias fused into the ScalarE exp together with the
     1/sqrt(d) scale: e = exp(0.125 * scores_T + pen[j]).
  3. masked key rows are dropped entirely (host gathers kept rows; a masked
     row's exp(-10000 + s) is exactly 0.0 in f32, so dropping it is exact).
  4. out_raw_T accumulated per head, PE-transposed back to natural layout,
     normalized by the row-sums, DMA'd out.
"""

import sys

sys.path.insert(0, "/opt/trn_rl_repo")

import numpy as np

import concourse.bass as bass
import concourse.bacc as bacc
import concourse.mybir as mybir
from concourse.tile import TileContext
from concourse.bass_utils import run_bass_kernel_spmd

B = 8
N = 1024          # sequence length (queries)
D = 1024          # model dim
H = 16            # heads
DH = 64           # head dim
NPAIR = H // 2    # head pairs (2 heads share one 128-row feature tile)
P = 128
F32 = mybir.dt.float32
F32R = mybir.dt.float32r
EXP = mybir.ActivationFunctionType.Exp

_CACHE = {}


def build_nc(n_j, repeat=0):
    """Build the per-core Bass graph.

    n_j: padded count of kept key rows (multiple of 128). If n_j == N the
         k/v projections read the full xT input (no separate gathered input).
    repeat: if > 0, wrap the whole compute in a For_i timing loop.

    Structure: projection matmul chains are interleaved into the attention
    jc-loops via a work feeder, so PE fills its exp-latency gaps with proj
    work instead of stalling (PE executes strictly in emission order).
    """
    n_jc = n_j // 128
    share_xt = n_j == N

    nc = bacc.Bacc(None, target_bir_lowering=False)
    xt_ext = nc.declare_dram_parameter("xt", [D, N], F32, isOutput=False)
    if not share_xt:
        xtkv_ext = nc.declare_dram_parameter("xtkv", [D, n_j], F32, isOutput=False)
    w_ext = nc.declare_dram_parameter("w", [D, 3 * D], F32, isOutput=False)
    pen_ext = nc.declare_dram_parameter("pen", [P, n_jc], F32, isOutput=False)
    id_ext = nc.declare_dram_parameter("ident", [P, P], F32, isOutput=False)
    out_ext = nc.declare_dram_parameter("out", [N, D], F32, isOutput=True)

    with TileContext(nc) as tc:
        with (
            tc.tile_pool(name="const", bufs=1) as const_pool,
            tc.tile_pool(name="xt", bufs=1) as xt_pool,
            tc.tile_pool(name="qk", bufs=1) as qk_pool,
            tc.tile_pool(name="vnat", bufs=1) as v_pool,
            tc.tile_pool(name="wq", bufs=3) as w_pool,
            tc.tile_pool(name="wv", bufs=1) as wv_pool,
            tc.tile_pool(name="e", bufs=2) as e_pool,
            tc.tile_pool(name="ot", bufs=4) as ot_pool,
            tc.tile_pool(name="ob", bufs=3) as ob_pool,
            tc.tile_pool(name="pss", bufs=1, space="PSUM") as pss_pool,
            tc.tile_pool(name="pso", bufs=1, space="PSUM") as pso_pool,
            tc.tile_pool(name="psj", bufs=2, space="PSUM") as psj_pool,
        ):
            pen_sb = const_pool.tile([P, n_jc], F32, tag="pen")
            nc.sync.dma_start(out=pen_sb[:], in_=pen_ext[:])
            id_sb = const_pool.tile([P, P], F32, tag="ident")
            nc.sync.dma_start(out=id_sb[:], in_=id_ext[:])

            xt_sb = [xt_pool.tile([P, N], F32R, tag=f"xt{dc}", name=f"xt{dc}")
                     for dc in range(8)]
            for h0 in range(0, N, 512):
                for dc in range(8):
                    nc.sync.dma_start(
                        out=xt_sb[dc][:, h0:h0 + 512],
                        in_=xt_ext[dc * P:(dc + 1) * P, h0:h0 + 512].bitcast(F32R),
                    )
                if h0 == 0:
                    # first q/k chains' weights right after the first xt halves
                    w_pre = {}
                    for fc_ in (0, 8):
                        wt_ = w_pool.tile([P, D], F32R, tag="w", name=f"wpre{fc_}")
                        for dc_ in range(8):
                            nc.sync.dma_start(
                                out=wt_[:, dc_ * P:(dc_ + 1) * P],
                                in_=w_ext[dc_ * P:(dc_ + 1) * P,
                                          fc_ * P:(fc_ + 1) * P].bitcast(F32R),
                            )
                        w_pre[fc_] = wt_
            if share_xt:
                xtkv_sb = xt_sb
            else:
                xtkv_sb = []
                for dc in range(8):
                    t = xt_pool.tile([P, n_j], F32R, tag=f"xtkv{dc}")
                    nc.sync.dma_start(
                        out=t[:, :], in_=xtkv_ext[dc * P:(dc + 1) * P, :].bitcast(F32R)
                    )
                    xtkv_sb.append(t)

            def body():
                qk_sb = [None] * 16
                v_nat = []
                for jc in range(n_jc):
                    t = v_pool.tile([P, H * 65], F32R, tag=f"v{jc}", name=f"v{jc}")
                    nc.vector.memset(
                        t.rearrange("p (h c) -> p h c", c=65)[:, :, 64:65].bitcast(F32),
                        1.0,
                    )
                    v_nat.append(t)
                wv_sb = {}

                # ---------- projection work units ----------
                def qk_chain(fc):
                    """Yield one closure per PE matmul for projection chain fc."""
                    n_cols = N if fc < 8 else n_j
                    src_ = xt_sb if fc < 8 else xtkv_sb
                    state = {}

                    def first():
                        if fc in w_pre:
                            w_sb = w_pre[fc]
                        else:
                            w_sb = w_pool.tile([P, D], F32R, tag="w", name=f"w{fc}")
                            for dc_ in range(8):
                                nc.sync.dma_start(
                                    out=w_sb[:, dc_ * P:(dc_ + 1) * P],
                                    in_=w_ext[dc_ * P:(dc_ + 1) * P,
                                              fc * P:(fc + 1) * P].bitcast(F32R),
                                )
                        ps = psj_pool.tile([P, N], F32, tag="proj", name=f"pj{fc}")
                        state["w"] = w_sb
                        state["ps"] = ps

                    halves = [(c0, min(c0 + 512, n_cols)) for c0 in range(0, n_cols, 512)]
                    units = [(hi, dc) for hi in range(len(halves)) for dc in range(8)]

                    def make(i, hi, dc):
                        def emit():
                            if i == 0:
                                first()
                            c0, c1 = halves[hi]
                            nc.tensor.matmul(
                                state["ps"][:, c0:c1],
                                state["w"][:, dc * P:(dc + 1) * P],
                                src_[dc][:, c0:c1],
                                start=(dc == 0), stop=(dc == 7),
                            )
                            if i == len(units) - 1:
                                dst = qk_pool.tile([P, n_cols], F32R,
                                                   tag=f"qk{fc}", name=f"qk{fc}")
                                nc.vector.tensor_copy(dst[:], state["ps"][:, :n_cols])
                                qk_sb[fc] = dst
                        return emit
                    return [make(i, hi, dc) for i, (hi, dc) in enumerate(units)]

                def wv_dma(hv):
                    def emit():
                        tiles = []
                        for dc_ in range(8):
                            t = wv_pool.tile([P, 512], F32R, tag=f"wv{hv}_{dc_}",
                                             name=f"wv{hv}_{dc_}")
                            nc.sync.dma_start(
                                out=t[:],
                                in_=w_ext[dc_ * P:(dc_ + 1) * P,
                                          2048 + hv * 512:2048 + (hv + 1) * 512
                                          ].bitcast(F32R),
                            )
                            tiles.append(t)
                        wv_sb[hv] = tiles
                    return emit

                def v_chain(hv, jc):
                    def make(dc):
                        def emit():
                            if dc == 0:
                                ps = psj_pool.tile([P, 512], F32, tag="proj",
                                                   name=f"pv{hv}_{jc}")
                                v_chain.ps = ps
                            nc.tensor.matmul(
                                v_chain.ps[:],
                                xtkv_sb[dc][:, jc * P:(jc + 1) * P],
                                wv_sb[hv][dc][:],
                                start=(dc == 0), stop=(dc == 7),
                            )
                            if dc == 7:
                                dstv = v_nat[jc].rearrange("p (h c) -> p h c", c=65)
                                nc.vector.tensor_copy(
                                    dstv[:, hv * 8:(hv + 1) * 8, 0:64],
                                    v_chain.ps[:].rearrange("p (h c) -> p h c", c=64),
                                )
                        return emit
                    return [make(dc) for dc in range(8)]

                # ---------- upfront: q0, k0, v(hv=0) ----------
                wv_dma(0)()
                for u in qk_chain(0):
                    u()
                for u in qk_chain(8):
                    u()
                for jc in range(n_jc):
                    for u in v_chain(0, jc):
                        u()

                # ---------- stream: remaining proj work, fed into attention ----
                stream = []
                markers = {}          # pair -> index into stream that must be done
                stream.append(wv_dma(1))
                # v(hv=1) chains must all be emitted before pair NPAIR//2
                # (the first consumer of head slots 8-15): spread over p=1..3.
                vq = list(range(n_jc))
                per_p = -(-len(vq) // 3)
                for p in range(1, NPAIR):
                    stream.extend(qk_chain(p))
                    stream.extend(qk_chain(8 + p))
                    if p <= 3:
                        for jc in vq[(p - 1) * per_p:p * per_p]:
                            stream.extend(v_chain(1, jc))
                    markers[p] = len(stream)
                pos = [0]
                extra = []            # epilogue PE units (fill work, no deadline)

                def feed(k, ne=0):
                    done = 0
                    while done < ne and extra:
                        extra.pop(0)()
                        done += 1
                    while done < k + ne:
                        if pos[0] < len(stream):
                            stream[pos[0]]()
                            pos[0] += 1
                        elif extra:
                            extra.pop(0)()
                        else:
                            return
                        done += 1

                def feed_until(idx):
                    while pos[0] < idx:
                        stream[pos[0]]()
                        pos[0] += 1

                # ---------- attention ----------
                def epilogue_units(p, ih, ot):
                    i0 = ih * 512
                    state = {}

                    def tr(ic, half):
                        def emit():
                            if "ps" not in state:
                                state["ps"] = psj_pool.tile(
                                    [P, 1024], F32, tag="proj", name=f"pt{p}_{ih}")
                            ps_t = state["ps"]
                            c = ic * P
                            t0 = ic * 256
                            if half == 0:
                                nc.tensor.transpose(
                                    ps_t[:, t0:t0 + 65],
                                    ot[:, c:c + P], id_sb[0:65, 0:65])
                            else:
                                nc.tensor.transpose(
                                    ps_t[:, t0 + 128:t0 + 193],
                                    ot[:, 512 + c:512 + c + P], id_sb[0:65, 0:65])
                            if ic == 3 and half == 1:
                                tail(ps_t)
                        return emit

                    def tail(ps_t):
                        for ic in range(4):
                            t0 = ic * 256
                            ob = ob_pool.tile([P, P], F32, tag="ob",
                                              name=f"ob{p}_{ih}_{ic}")
                            rc = ob_pool.tile([P, 2], F32, tag="rc",
                                              name=f"rc{p}_{ih}_{ic}")
                            nc.vector.reciprocal(rc[:, 0:1], ps_t[:, t0 + 64:t0 + 65])
                            nc.vector.reciprocal(rc[:, 1:2], ps_t[:, t0 + 192:t0 + 193])
                            nc.vector.tensor_scalar_mul(
                                ob[:, 0:64], ps_t[:, t0:t0 + 64], rc[:, 0:1]
                            )
                            nc.vector.tensor_scalar_mul(
                                ob[:, 64:128], ps_t[:, t0 + 128:t0 + 192], rc[:, 1:2]
                            )
                            nc.sync.dma_start(
                                out=out_ext[i0 + ic * P:i0 + (ic + 1) * P,
                                            p * P:(p + 1) * P],
                                in_=ob[:],
                            )

                    return [tr(ic, h) for ic in range(4) for h in range(2)]

                for p in range(NPAIR):
                    if p in markers:
                        feed_until(markers[p])
                    qT = qk_sb[p]
                    kT = qk_sb[8 + p]
                    ha, hb = 2 * p, 2 * p + 1
                    for ih in range(2):
                        i0 = ih * 512
                        ps_s = pss_pool.tile([P, 1024], F32, tag="s",
                                             name=f"s{p}_{ih}")
                        ps_o = pso_pool.tile([65, 1024], F32, tag="o",
                                             name=f"o{p}_{ih}")
                        for jc in range(n_jc):
                            nc.tensor.matmul(
                                ps_s[:, 0:512],
                                kT[0:64, jc * P:(jc + 1) * P],
                                qT[0:64, i0:i0 + 512],
                                start=True, stop=True,
                            )
                            nc.tensor.matmul(
                                ps_s[:, 512:1024],
                                kT[64:128, jc * P:(jc + 1) * P],
                                qT[64:128, i0:i0 + 512],
                                start=True, stop=True,
                            )
                            e_sb = e_pool.tile([P, 1024], F32R, tag="e",
                                               name=f"e{p}_{ih}_{jc}")
                            nc.scalar.activation(
                                e_sb[:], ps_s[:], EXP,
                                bias=pen_sb[:, jc:jc + 1], scale=0.125,
                            )
                            nc.tensor.matmul(
                                ps_o[:, 0:512],
                                v_nat[jc][:, ha * 65:(ha + 1) * 65],
                                e_sb[:, 0:512],
                                start=(jc == 0), stop=(jc == n_jc - 1),
                            )
                            nc.tensor.matmul(
                                ps_o[:, 512:1024],
                                v_nat[jc][:, hb * 65:(hb + 1) * 65],
                                e_sb[:, 512:1024],
                                start=(jc == 0), stop=(jc == n_jc - 1),
                            )
                            feed(3)
                        ot = ot_pool.tile([65, 1024], F32, tag="ot",
                                          name=f"ot{p}_{ih}")
                        nc.vector.tensor_copy(ot[:], ps_o[:])
                        while len(extra) >= 8:   # emit previous block's epilogue
                            extra.pop(0)()
                        extra.extend(epilogue_units(p, ih, ot))
                feed(10 ** 9)
                while extra:
                    extra.pop(0)()

            if repeat > 0:
                with tc.For_i(0, repeat, 1):
                    body()
            else:
                body()

    nc.compile()
    return nc


def _host_prep(x, mask, w_qkv):
    """Shard + lay out inputs per core. Returns (in_maps, n_j)."""
    x = np.ascontiguousarray(x, dtype=np.float32)
    mask = np.asarray(mask)
    w_qkv = np.ascontiguousarray(w_qkv, dtype=np.float32)

    # kept key rows per batch: j=0 always kept, then mask over rows 1..N-1
    keep = np.concatenate([np.ones((B, 1), dtype=bool), mask.astype(bool)], axis=1)
    counts = keep.sum(axis=1)
    n_j = int(np.ceil(counts.max() / 128.0) * 128)
    n_j = min(n_j, N)

    ident = np.eye(P, dtype=np.float32)
    in_maps = []
    for b in range(B):
        xt = np.ascontiguousarray(x[b].T)               # [D, N]
        idx = np.nonzero(keep[b])[0]
        m = {"xt": xt, "w": w_qkv, "ident": ident}
        pen = np.full(n_j, -10000.0, dtype=np.float32)  # padding rows masked out
        pen[: len(idx)] = 0.0
        m["pen"] = np.ascontiguousarray(pen.reshape(n_j // 128, 128).T)  # [128, n_jc]
        if n_j == N:
            # no gather: full rows, penalty by original position
            penf = np.full(N, -10000.0, dtype=np.float32)
            penf[keep[b]] = 0.0
            m["pen"] = np.ascontiguousarray(penf.reshape(N // 128, 128).T)
        else:
            xkv = np.zeros((D, n_j), dtype=np.float32)
            xkv[:, : len(idx)] = xt[:, idx]
            m["xtkv"] = xkv
        in_maps.append(m)
    return in_maps, n_j


def kernel(x, mask, w_qkv):
    in_maps, n_j = _host_prep(x, mask, w_qkv)
    if n_j not in _CACHE:
        _CACHE[n_j] = build_nc(n_j)
    nc = _CACHE[n_j]
    res = run_bass_kernel_spmd(nc, in_maps, core_ids=list(range(B)))
    out = np.stack([np.asarray(res.results[i]["out"]) for i in range(B)], axis=0)
    return out.astype(np.float32)


if __name__ == "__main__":
    rng = np.random.default_rng(0)
    x = rng.standard_normal((B, N, D), dtype=np.float32)
    mask = rng.integers(0, 2, size=(B, N - 1)).astype(np.int32)
    w = (rng.standard_normal((D, 3 * D), dtype=np.float32) * D ** -0.5).astype(np.float32)
    out = kernel(x=x, mask=mask, w_qkv=w)
    print("out", out.shape, out.dtype, float(np.abs(out).mean()))
